# revision 3
# baseline (speedup 1.0000x reference)
"""MinGRU (parallel log-space scan) Trainium2 Bass kernel.

Problem (hardcoded):
    x:    [B=8, S=4096, D=1024] f32
    W_hg: [D=1024, 2*D=2048]    f32
    out:  [B=8, S=4096, D=1024] f32

    hg = x @ W_hg ; hidden, gate = split(hg)
    h_t = (1-z_t) * h_{t-1} + z_t * g(hidden_t),  z = sigmoid(gate),
    g(v) = v + 0.5 if v >= 0 else sigmoid(v)  ==  max(v + 0.5, sigmoid(v))

Sharding: data-parallel over batch, one batch row per NeuronCore (8 cores),
W_hg replicated.

Layout strategy: the scan must run along the free dimension (channels on
partitions), so the device works entirely in the transposed layout
hg^T/h^T = [channels, seq]. The host packs x per batch row into
per-chunk-contiguous bf16 blocks and W into per-k-slice-contiguous bf16
blocks (gate half first within each slice) so every SBUF load is a single
DMA instruction (the Sync engine serializes DMA issues at ~0.6us each).

bf16 matmuls: 1 cyc/row on the PE like fp32r, but FWL (fast weight load)
hides the LDWEIGHTS stream behind the matmuls, and the x/W DMA volume
halves. Accuracy: ~2.3e-3 max rel err, far below the 2e-2 gate.
(fp8 DoubleRow was evaluated: e4m3 quantization of x/W puts the final
max rel err at 3.5e-2 — over the gate — so the PE floor is bf16's
1 col/cyc.)

Per-core pipeline over seq chunks of C=512:
  one DMA for the x^T chunk block [128, 8j x C] (bf16)
  -> per k: bf16 matmuls gate then hidden, accumulated in PSUM
     (a = sigmoid(-gate) on ACT overlaps the hidden matmuls)
  -> DVE: gh = (hidden + 0.5) max sigh ; bneg = (a - 1) * gh
  -> DVE: h = scan(a * h_prev) - bneg   (carry chained across chunks)
  -> DMA h^T tile straight to DRAM out^T.

Head: the aggregate DMA rate is already at the ~360 GB/s HBM cap, so the
head is critical-BYTE bound. Chunk 0's x block is loaded as three DMAs
(j0-1, j2-4, j5-7) and W k-slice 0 as two (gate half via the Scalar
engine's DGE queue, in parallel with Sync), so the first matmul group
starts after ~0.5MB instead of 1.5MB, with Tile's range-precise deps
letting each j matmul wait only for its own piece. PE p-state warmup
matmuls on a never-written (garbage) tile bridge the wait so the
0.65->2.4 GHz clock ramp overlaps the head. A 4-byte fence DMA reading
the tail of the last x0 piece holds the non-critical loads back until
chunk 0 has fully landed.

Tail: the last two k-tiles' hidden accumulation is split and the
pointwise/scan/store runs in pieces ([256,256] for k6, [256,128,128]
for k7) so the final stores overlap the final scans; k7's stores issue
from the (idle) Scalar DGE queue to dodge Sync's 0.6us issue
serialization.
"""

import numpy as np

import concourse.bacc as bacc
import concourse.tile as tile
from concourse import mybir

B, S, D = 8, 4096, 1024
N_CORES = 8
P = 128  # partitions
# Seq chunk schedule: uniform 512 (the PSUM-bank maximum). Smaller lead-in
# chunks were tried and lose: the extra matmul instructions and pipeline
# gaps cost more than the smaller critical head DMA saves.
CHUNKS = [512] * 8
CHUNK_OFF = [sum(CHUNKS[:i]) for i in range(len(CHUNKS))]
assert sum(CHUNKS) == S
N_DT = D // P  # 8 d-tiles (contraction)
N_KT = D // P  # 8 output channel tiles (hidden dim = D)
WBLK = N_DT * 2 * P  # packed w k-slice columns (gate half then hidden half)
HALF = N_DT * P  # columns per half (gate or hidden) of a w k-slice

F32 = mybir.dt.float32
BF16 = mybir.dt.bfloat16
MM_DT = BF16

_COMPILED = {}


def _build():
    nc = bacc.Bacc(
        "TRN2", target_bir_lowering=False, debug=False, num_devices=N_CORES
    )
    # packed layouts (see make_in_maps): one contiguous run per SBUF load
    xt_d = nc.dram_tensor(
        "xt", [P, N_DT * S], MM_DT, kind="ExternalInput"
    ).ap()
    w_d = nc.dram_tensor(
        "w", [P, N_KT * WBLK], MM_DT, kind="ExternalInput"
    ).ap()
    out_d = nc.dram_tensor("outT", [D, S], F32, kind="ExternalOutput").ap()

    AL = mybir.AluOpType
    SIG = mybir.ActivationFunctionType.Sigmoid

    with tile.TileContext(nc) as tc:
        with (
            tc.tile_pool(name="wpool", bufs=1) as wpool,
            tc.tile_pool(name="xtp", bufs=3) as xt_pool,
            tc.tile_pool(name="pw", bufs=3) as pw_pool,
            tc.tile_pool(name="hp", bufs=3) as h_pool,
            tc.tile_pool(name="pshg", bufs=8, space="PSUM") as psum_hg,
        ):
            w_tile = wpool.tile([P, N_KT * WBLK], MM_DT, name="w_tile")

            def wload(k):
                nc.sync.dma_start(
                    w_tile[:, k * WBLK : (k + 1) * WBLK],
                    w_d[:, k * WBLK : (k + 1) * WBLK],
                )

            def load_x_chunk(sc, name):
                csz = CHUNKS[sc]
                off = N_DT * CHUNK_OFF[sc]
                t = xt_pool.tile([P, N_DT * csz], MM_DT, tag="xc", name=name)
                nc.sync.dma_start(t[:], xt_d[:, off : off + N_DT * csz])
                return t

            # PE p-state warmup: the tensor engine ramps 0.65->2.4 GHz over
            # ~3us of continuous execution. Run garbage matmuls on a
            # memset tile (PSUM never read) while the first real DMAs are
            # in flight so the ramp cost overlaps the head instead of the
            # real stream. Two memsets on different engines so the first
            # LDWEIGHTS only waits for the small Vector one.
            warm = xt_pool.tile([P, 512], MM_DT, tag="warm", bufs=1)
            nc.vector.memset(warm[:, 0:P], 0.0)
            nc.gpsimd.memset(warm[:, P:512], 0.0)
            warm_ps = psum_hg.tile([P, 512], F32, tag="ph")
            for i in range(8):
                nc.tensor.matmul(
                    warm_ps[:], warm[:, 0:P], warm[:],
                    start=(i == 0), stop=(i == 7),
                )

            # Critical path first, finest useful granularity: chunk 0 of
            # x^T in three pieces on Sync's DGE queue, W k-slice 0's two
            # halves on Scalar's DGE queue in parallel. The first gate
            # matmul needs only x0[j0-1] + the gate w half (~0.5MB).
            CS0 = CHUNKS[0]
            x0 = xt_pool.tile([P, N_DT * CS0], MM_DT, tag="xc", name="x0")
            nc.sync.dma_start(x0[:, 0 : 2 * CS0], xt_d[:, 0 : 2 * CS0])
            nc.scalar.dma_start(w_tile[:, 0:HALF], w_d[:, 0:HALF])
            nc.sync.dma_start(
                x0[:, 2 * CS0 : 5 * CS0], xt_d[:, 2 * CS0 : 5 * CS0]
            )
            nc.sync.dma_start(
                x0[:, 5 * CS0 : 8 * CS0], xt_d[:, 5 * CS0 : 8 * CS0]
            )
            nc.scalar.dma_start(w_tile[:, HALF:WBLK], w_d[:, HALF:WBLK])
            # Hold back the non-critical loads until x0 has landed so they
            # don't steal DMA bandwidth from it: this 4-byte DMA reads the
            # tail of x0's last piece, so the in-order Sync engine blocks
            # here until the critical transfers complete.
            fence = xt_pool.tile([P, 2], MM_DT, tag="fence", bufs=1)
            nc.sync.dma_start(fence[0:1, 0:2], x0[0:1, 8 * CS0 - 2 : 8 * CS0])
            wload(1)
            x1 = load_x_chunk(1, "x1")
            for k in range(2, N_KT):
                wload(k)

            # lhsT slices: w_sb[kk][j]; kk in [0,8) hidden, [8,16) gate
            # (packed gate-half-first within each k-slice)
            w_sb = [
                [
                    w_tile[
                        :,
                        k * WBLK + (HALF if b == 0 else 0) + j * P :
                        k * WBLK + (HALF if b == 0 else 0) + (j + 1) * P,
                    ]
                    for j in range(N_DT)
                ]
                for b in range(2)
                for k in range(N_KT)
            ]

            prev_h = [None] * N_KT
            for sc, csz in enumerate(CHUNKS):
                s0 = CHUNK_OFF[sc]
                last_chunk = sc == len(CHUNKS) - 1
                if sc == 0:
                    xts = x0
                elif sc == 1:
                    xts = x1
                else:
                    xts = load_x_chunk(sc, None)

                def mm_group(ps, kk, lo, hi):
                    for j in range(N_DT):
                        nc.tensor.matmul(
                            ps[:],
                            w_sb[kk][j],
                            xts[:, j * csz + lo : j * csz + hi],
                            start=(j == 0),
                            stop=(j == N_DT - 1),
                        )

                for k in range(N_KT):
                    # split the hidden accumulation and pointwise for the
                    # last TWO k-tiles: k6's DVE work otherwise backlogs
                    # into k7's matmul window and extends the tail drain.
                    # k7 ends with two 128-col pieces so the very last
                    # scan->store chain is short.
                    if last_chunk and k == N_KT - 2:
                        pieces = [csz // 2, csz // 2]
                    elif last_chunk and k == N_KT - 1:
                        pieces = [csz // 2, csz // 4, csz // 4]
                    else:
                        pieces = [csz]
                    last_k = len(pieces) > 1
                    # gate first: a = sigmoid(-gate) is ready while the
                    # hidden matmuls run, shortening the per-k tail chain
                    pg = psum_hg.tile([P, csz], F32, tag="ph")  # gate
                    mm_group(pg, N_KT + k, 0, csz)
                    a_t = pw_pool.tile([P, csz], F32, tag="a")
                    nc.scalar.activation(a_t[:], pg[:], SIG, scale=-1.0)
                    if last_k:
                        # split accumulation (separate PSUM banks: a start
                        # flag zeroes the whole 2KB zero-region) so the
                        # pointwise tail starts before the final matmul
                        phs = []
                        off = 0
                        for psz in pieces:
                            ph_p = psum_hg.tile([P, psz], F32, tag="ph")
                            for j in range(N_DT):
                                nc.tensor.matmul(
                                    ph_p[:], w_sb[k][j],
                                    xts[:, j * csz + off : j * csz + off + psz],
                                    start=(j == 0), stop=(j == N_DT - 1),
                                )
                            phs.append((off, off + psz, ph_p))
                            off += psz

                        def ph_piece(lo, hi):
                            for (plo, phi, ph_p) in phs:
                                if lo >= plo and hi <= phi:
                                    return ph_p[:, lo - plo : hi - plo]
                            raise AssertionError((lo, hi))
                    else:
                        ph = psum_hg.tile([P, csz], F32, tag="ph")  # hidden
                        mm_group(ph, k, 0, csz)

                        def ph_piece(lo, hi):
                            return ph[:, lo:hi]

                    # pointwise/scan pieces match the hidden PSUM pieces
                    # (finer splits lose to per-instruction overhead since
                    # all three pointwise ops serialize on the DVE)
                    sigh = pw_pool.tile([P, csz], F32, tag="sigh")
                    gh = pw_pool.tile([P, csz], F32, tag="gh")
                    bneg = pw_pool.tile([P, csz], F32, tag="bneg")
                    h = h_pool.tile([P, csz], F32, tag=f"h{k}")
                    lo = 0
                    for psz in pieces:
                        lo, hi = lo, lo + psz
                        php = ph_piece(lo, hi)
                        # sigh = sigmoid(hidden)
                        nc.scalar.activation(sigh[:, lo:hi], php, SIG)
                        # g(hidden) = max(hidden + 0.5, sigmoid(hidden))
                        nc.vector.scalar_tensor_tensor(
                            gh[:, lo:hi], php, 0.5, sigh[:, lo:hi],
                            op0=AL.add, op1=AL.max,
                        )
                        # bneg = (a - 1) * g = -(z * g)
                        nc.vector.scalar_tensor_tensor(
                            bneg[:, lo:hi], a_t[:, lo:hi], 1.0, gh[:, lo:hi],
                            op0=AL.subtract, op1=AL.mult,
                        )
                        # h_t = a_t * h_{t-1} - bneg_t  (linear recurrence)
                        if lo == 0:
                            init = (
                                0.0
                                if prev_h[k] is None
                                else prev_h[k][:, -1:]
                            )
                        else:
                            init = h[:, lo - 1 : lo]
                        nc.vector.tensor_tensor_scan(
                            h[:, lo:hi], a_t[:, lo:hi], bneg[:, lo:hi], init,
                            op0=AL.mult, op1=AL.subtract,
                        )
                        # k7's tail stores go via the idle Scalar DGE queue
                        # so they don't serialize behind Sync's issues
                        eng = (
                            nc.scalar
                            if (last_chunk and k == N_KT - 1)
                            else nc.sync
                        )
                        eng.dma_start(
                            out_d[k * P : (k + 1) * P, s0 + lo : s0 + hi],
                            h[:, lo:hi],
                        )
                        lo = hi
                    prev_h[k] = h
    nc.compile()
    return nc


def _get_nc():
    key = str(MM_DT)
    if key not in _COMPILED:
        _COMPILED[key] = _build()
    return _COMPILED[key]


def make_in_maps(x: np.ndarray, W_hg: np.ndarray) -> list[dict]:
    import ml_dtypes

    bf = ml_dtypes.bfloat16
    x = np.asarray(x, dtype=np.float32)
    w = np.asarray(W_hg, dtype=np.float32)

    # x pack: per-chunk contiguous blocks [p, sc][j, t]
    def pack_x(xb):
        blocks = []
        for sc, csz in enumerate(CHUNKS):
            s0 = CHUNK_OFF[sc]
            blk = xb[s0 : s0 + csz, :]  # [csz, D]
            blocks.append(
                blk.reshape(csz, N_DT, P).transpose(2, 1, 0).reshape(P, -1)
            )
        return np.ascontiguousarray(np.concatenate(blocks, axis=1).astype(bf))

    xp = [pack_x(x[b]) for b in range(N_CORES)]
    # w pack: W[j*128+p, b*1024 + k*128 + c] -> wp[p, k, half, j, c]
    # with half 0 = gate (b=1), half 1 = hidden (b=0)
    wp = np.ascontiguousarray(
        w.reshape(N_DT, P, 2, N_KT, P)
        .transpose(1, 3, 2, 0, 4)[:, :, ::-1]
        .reshape(P, N_KT * WBLK)
        .astype(bf)
    )
    return [{"xt": xp[b], "w": wp} for b in range(N_CORES)]


def kernel(x: np.ndarray, W_hg: np.ndarray) -> np.ndarray:
    from concourse.bass_utils import run_bass_kernel_spmd

    assert x.shape == (B, S, D) and W_hg.shape == (D, 2 * D)
    nc = _get_nc()
    in_maps = make_in_maps(x, W_hg)
    res = run_bass_kernel_spmd(nc, in_maps, list(range(N_CORES)))
    out = np.empty((B, S, D), dtype=np.float32)
    for b in range(N_CORES):
        out[b] = res.results[b]["outT"].T
    return out


# revision 6
# speedup vs baseline: 1.0062x; 1.0062x over previous
"""MinGRU (parallel log-space scan) Trainium2 Bass kernel.

Problem (hardcoded):
    x:    [B=8, S=4096, D=1024] f32
    W_hg: [D=1024, 2*D=2048]    f32
    out:  [B=8, S=4096, D=1024] f32

    hg = x @ W_hg ; hidden, gate = split(hg)
    h_t = (1-z_t) * h_{t-1} + z_t * g(hidden_t),  z = sigmoid(gate),
    g(v) = v + 0.5 if v >= 0 else sigmoid(v)  ==  max(v + 0.5, sigmoid(v))

Sharding: data-parallel over batch, one batch row per NeuronCore (8 cores),
W_hg replicated.

Layout strategy: the scan must run along the free dimension (channels on
partitions), so the device works entirely in the transposed layout
hg^T/h^T = [channels, seq]. The host packs x per batch row into
per-chunk-contiguous bf16 blocks and W into per-k-slice-contiguous bf16
blocks (gate half first within each slice) so every SBUF load is a single
DMA instruction (the Sync engine serializes DMA issues at ~0.6us each).

bf16 matmuls: 1 cyc/row on the PE like fp32r, but FWL (fast weight load)
hides the LDWEIGHTS stream behind the matmuls, and the x/W DMA volume
halves. Accuracy: ~2.3e-3 max rel err, far below the 2e-2 gate.
(fp8 DoubleRow was evaluated: e4m3 quantization of x/W puts the final
max rel err at 3.5e-2 — over the gate — so the PE floor is bf16's
1 col/cyc.)

Per-core pipeline over seq chunks of C=512:
  one DMA for the x^T chunk block [128, 8j x C] (bf16)
  -> per k: bf16 matmuls gate then hidden, accumulated in PSUM
     (a = sigmoid(-gate) on ACT overlaps the hidden matmuls)
  -> DVE: gh = (hidden + 0.5) max sigh ; bneg = (a - 1) * gh
  -> DVE: h = scan(a * h_prev) - bneg   (carry chained across chunks)
  -> DMA h^T tile straight to DRAM out^T.

Head: the aggregate DMA rate is already at the ~360 GB/s HBM cap, so the
head is critical-BYTE bound. Chunk 0's x block is loaded as three DMAs
(j0-1, j2-4, j5-7) and W k-slice 0 as two (gate half via the Scalar
engine's DGE queue, in parallel with Sync), so the first matmul group
starts after ~0.5MB instead of 1.5MB, with Tile's range-precise deps
letting each j matmul wait only for its own piece. PE p-state warmup
matmuls on a never-written (garbage) tile bridge the wait so the
0.65->2.4 GHz clock ramp overlaps the head. A 4-byte fence DMA reading
the tail of the last x0 piece holds the non-critical loads back until
chunk 0 has fully landed.

Tail: the last two k-tiles' hidden accumulation is split and the
pointwise/scan/store runs in pieces ([256,256] for k6, [256,128,128]
for k7) so the final stores overlap the final scans; k7's stores issue
from the (idle) Scalar DGE queue to dodge Sync's 0.6us issue
serialization.
"""

import numpy as np

import concourse.bacc as bacc
import concourse.tile as tile
from concourse import mybir

B, S, D = 8, 4096, 1024
N_CORES = 8
P = 128  # partitions
# Seq chunk schedule: uniform 512 (the PSUM-bank maximum). Smaller lead-in
# chunks were tried and lose: the extra matmul instructions and pipeline
# gaps cost more than the smaller critical head DMA saves.
CHUNKS = [512] * 8
CHUNK_OFF = [sum(CHUNKS[:i]) for i in range(len(CHUNKS))]
assert sum(CHUNKS) == S
N_DT = D // P  # 8 d-tiles (contraction)
N_KT = D // P  # 8 output channel tiles (hidden dim = D)
WBLK = N_DT * 2 * P  # packed w k-slice columns (gate half then hidden half)
HALF = N_DT * P  # columns per half (gate or hidden) of a w k-slice

F32 = mybir.dt.float32
BF16 = mybir.dt.bfloat16
MM_DT = BF16

_COMPILED = {}


def _build():
    nc = bacc.Bacc(
        "TRN2", target_bir_lowering=False, debug=False, num_devices=N_CORES
    )
    # packed layouts (see make_in_maps): one contiguous run per SBUF load
    xt_d = nc.dram_tensor(
        "xt", [P, N_DT * S], MM_DT, kind="ExternalInput"
    ).ap()
    w_d = nc.dram_tensor(
        "w", [P, N_KT * WBLK], MM_DT, kind="ExternalInput"
    ).ap()
    out_d = nc.dram_tensor("outT", [D, S], F32, kind="ExternalOutput").ap()

    AL = mybir.AluOpType
    SIG = mybir.ActivationFunctionType.Sigmoid

    with tile.TileContext(nc) as tc:
        with (
            tc.tile_pool(name="wpool", bufs=1) as wpool,
            tc.tile_pool(name="xtp", bufs=3) as xt_pool,
            tc.tile_pool(name="pw", bufs=3) as pw_pool,
            tc.tile_pool(name="hp", bufs=3) as h_pool,
            tc.tile_pool(name="pshg", bufs=8, space="PSUM") as psum_hg,
        ):
            w_tile = wpool.tile([P, N_KT * WBLK], MM_DT, name="w_tile")

            def wload(k):
                nc.sync.dma_start(
                    w_tile[:, k * WBLK : (k + 1) * WBLK],
                    w_d[:, k * WBLK : (k + 1) * WBLK],
                )

            def load_x_chunk(sc, name):
                csz = CHUNKS[sc]
                off = N_DT * CHUNK_OFF[sc]
                t = xt_pool.tile([P, N_DT * csz], MM_DT, tag="xc", name=name)
                nc.sync.dma_start(t[:], xt_d[:, off : off + N_DT * csz])
                return t

            # PE p-state warmup: the tensor engine ramps 0.65->2.4 GHz over
            # ~3us of continuous execution. Run garbage matmuls on a
            # memset tile (PSUM never read) while the first real DMAs are
            # in flight so the ramp cost overlaps the head instead of the
            # real stream. Two memsets on different engines so the first
            # LDWEIGHTS only waits for the small Vector one.
            warm = xt_pool.tile([P, 512], MM_DT, tag="warm", bufs=1)
            nc.vector.memset(warm[:, 0:P], 0.0)
            nc.gpsimd.memset(warm[:, P:512], 0.0)
            warm_ps = psum_hg.tile([P, 512], F32, tag="ph")
            for i in range(11):
                nc.tensor.matmul(
                    warm_ps[:], warm[:, 0:P], warm[:],
                    start=(i == 0), stop=(i == 10),
                )

            # Critical path: a single FIFO train on Sync's DGE queue so
            # the 16 DMA rings stay deep enough to pipeline the ~1.3us
            # HBM read latency (small isolated transfers are latency-
            # bound at ~2 packets/ring). Ring FIFO order makes pieces
            # complete in issue order: gate w half of k0 first, then x0
            # in two halves, then the hidden w half (needed ~2.3us later
            # than the gate one).
            CS0 = CHUNKS[0]
            x0 = xt_pool.tile([P, N_DT * CS0], MM_DT, tag="xc", name="x0")
            nc.sync.dma_start(w_tile[:, 0:HALF], w_d[:, 0:HALF])
            nc.sync.dma_start(x0[:, 0 : 4 * CS0], xt_d[:, 0 : 4 * CS0])
            nc.sync.dma_start(
                x0[:, 4 * CS0 : 8 * CS0], xt_d[:, 4 * CS0 : 8 * CS0]
            )
            nc.sync.dma_start(w_tile[:, HALF:WBLK], w_d[:, HALF:WBLK])
            # Hold back the non-critical loads until x0 has landed so they
            # don't steal DMA bandwidth from it: this 4-byte DMA reads the
            # tail of x0's last piece, so the in-order Sync engine blocks
            # here until the critical transfers complete.
            fence = xt_pool.tile([P, 2], MM_DT, tag="fence", bufs=1)
            nc.sync.dma_start(fence[0:1, 0:2], x0[0:1, 8 * CS0 - 2 : 8 * CS0])
            wload(1)
            x1 = load_x_chunk(1, "x1")
            for k in range(2, N_KT):
                wload(k)

            # lhsT slices: w_sb[kk][j]; kk in [0,8) hidden, [8,16) gate
            # (packed gate-half-first within each k-slice)
            w_sb = [
                [
                    w_tile[
                        :,
                        k * WBLK + (HALF if b == 0 else 0) + j * P :
                        k * WBLK + (HALF if b == 0 else 0) + (j + 1) * P,
                    ]
                    for j in range(N_DT)
                ]
                for b in range(2)
                for k in range(N_KT)
            ]

            prev_h = [None] * N_KT
            for sc, csz in enumerate(CHUNKS):
                s0 = CHUNK_OFF[sc]
                last_chunk = sc == len(CHUNKS) - 1
                if sc == 0:
                    xts = x0
                elif sc == 1:
                    xts = x1
                else:
                    xts = load_x_chunk(sc, None)

                def mm_group(ps, kk, lo, hi):
                    for j in range(N_DT):
                        nc.tensor.matmul(
                            ps[:],
                            w_sb[kk][j],
                            xts[:, j * csz + lo : j * csz + hi],
                            start=(j == 0),
                            stop=(j == N_DT - 1),
                        )

                for k in range(N_KT):
                    # split the hidden accumulation and pointwise for the
                    # last TWO k-tiles: k6's DVE work otherwise backlogs
                    # into k7's matmul window and extends the tail drain.
                    # (Finer pieces than halves lose: the DVE's ~250ns
                    # per-instruction overhead bloats the serial tail
                    # chain more than the smaller last piece saves.)
                    if last_chunk and k >= N_KT - 2:
                        pieces = [csz // 2, csz // 2]
                    else:
                        pieces = [csz]
                    last_k = len(pieces) > 1
                    # gate first: a = sigmoid(-gate) is ready while the
                    # hidden matmuls run, shortening the per-k tail chain
                    pg = psum_hg.tile([P, csz], F32, tag="ph")  # gate
                    mm_group(pg, N_KT + k, 0, csz)
                    a_t = pw_pool.tile([P, csz], F32, tag="a")
                    nc.scalar.activation(a_t[:], pg[:], SIG, scale=-1.0)
                    if last_k:
                        # split accumulation (separate PSUM banks: a start
                        # flag zeroes the whole 2KB zero-region) so the
                        # pointwise tail starts before the final matmul
                        phs = []
                        off = 0
                        for psz in pieces:
                            ph_p = psum_hg.tile([P, psz], F32, tag="ph")
                            for j in range(N_DT):
                                nc.tensor.matmul(
                                    ph_p[:], w_sb[k][j],
                                    xts[:, j * csz + off : j * csz + off + psz],
                                    start=(j == 0), stop=(j == N_DT - 1),
                                )
                            phs.append((off, off + psz, ph_p))
                            off += psz

                        def ph_piece(lo, hi):
                            for (plo, phi, ph_p) in phs:
                                if lo >= plo and hi <= phi:
                                    return ph_p[:, lo - plo : hi - plo]
                            raise AssertionError((lo, hi))
                    else:
                        ph = psum_hg.tile([P, csz], F32, tag="ph")  # hidden
                        mm_group(ph, k, 0, csz)

                        def ph_piece(lo, hi):
                            return ph[:, lo:hi]

                    # pointwise/scan pieces match the hidden PSUM pieces
                    # (finer splits lose to per-instruction overhead since
                    # all three pointwise ops serialize on the DVE)
                    sigh = pw_pool.tile([P, csz], F32, tag="sigh")
                    gh = pw_pool.tile([P, csz], F32, tag="gh")
                    bneg = pw_pool.tile([P, csz], F32, tag="bneg")
                    h = h_pool.tile([P, csz], F32, tag=f"h{k}")
                    lo = 0
                    for psz in pieces:
                        lo, hi = lo, lo + psz
                        php = ph_piece(lo, hi)
                        # sigh = sigmoid(hidden)
                        nc.scalar.activation(sigh[:, lo:hi], php, SIG)
                        # g(hidden) = max(hidden + 0.5, sigmoid(hidden))
                        nc.vector.scalar_tensor_tensor(
                            gh[:, lo:hi], php, 0.5, sigh[:, lo:hi],
                            op0=AL.add, op1=AL.max,
                        )
                        # bneg = (a - 1) * g = -(z * g)
                        nc.vector.scalar_tensor_tensor(
                            bneg[:, lo:hi], a_t[:, lo:hi], 1.0, gh[:, lo:hi],
                            op0=AL.subtract, op1=AL.mult,
                        )
                        # h_t = a_t * h_{t-1} - bneg_t  (linear recurrence)
                        if lo == 0:
                            init = (
                                0.0
                                if prev_h[k] is None
                                else prev_h[k][:, -1:]
                            )
                        else:
                            init = h[:, lo - 1 : lo]
                        nc.vector.tensor_tensor_scan(
                            h[:, lo:hi], a_t[:, lo:hi], bneg[:, lo:hi], init,
                            op0=AL.mult, op1=AL.subtract,
                        )
                        nc.sync.dma_start(
                            out_d[k * P : (k + 1) * P, s0 + lo : s0 + hi],
                            h[:, lo:hi],
                        )
                        lo = hi
                    prev_h[k] = h
    nc.compile()
    return nc


def _get_nc():
    key = str(MM_DT)
    if key not in _COMPILED:
        _COMPILED[key] = _build()
    return _COMPILED[key]


def make_in_maps(x: np.ndarray, W_hg: np.ndarray) -> list[dict]:
    import ml_dtypes

    bf = ml_dtypes.bfloat16
    x = np.asarray(x, dtype=np.float32)
    w = np.asarray(W_hg, dtype=np.float32)

    # x pack: per-chunk contiguous blocks [p, sc][j, t]
    def pack_x(xb):
        blocks = []
        for sc, csz in enumerate(CHUNKS):
            s0 = CHUNK_OFF[sc]
            blk = xb[s0 : s0 + csz, :]  # [csz, D]
            blocks.append(
                blk.reshape(csz, N_DT, P).transpose(2, 1, 0).reshape(P, -1)
            )
        return np.ascontiguousarray(np.concatenate(blocks, axis=1).astype(bf))

    xp = [pack_x(x[b]) for b in range(N_CORES)]
    # w pack: W[j*128+p, b*1024 + k*128 + c] -> wp[p, k, half, j, c]
    # with half 0 = gate (b=1), half 1 = hidden (b=0)
    wp = np.ascontiguousarray(
        w.reshape(N_DT, P, 2, N_KT, P)
        .transpose(1, 3, 2, 0, 4)[:, :, ::-1]
        .reshape(P, N_KT * WBLK)
        .astype(bf)
    )
    return [{"xt": xp[b], "w": wp} for b in range(N_CORES)]


def kernel(x: np.ndarray, W_hg: np.ndarray) -> np.ndarray:
    from concourse.bass_utils import run_bass_kernel_spmd

    assert x.shape == (B, S, D) and W_hg.shape == (D, 2 * D)
    nc = _get_nc()
    in_maps = make_in_maps(x, W_hg)
    res = run_bass_kernel_spmd(nc, in_maps, list(range(N_CORES)))
    out = np.empty((B, S, D), dtype=np.float32)
    for b in range(N_CORES):
        out[b] = res.results[b]["outT"].T
    return out
